# revision 3
# baseline (speedup 1.0000x reference)
"""MoE layer (E=8 experts, top-2, swiglu FFN) on 8 Trainium2 NeuronCores.

Strategy: expert dispatch on host + tensor-parallel-over-hidden on device.
  - Router (logits -> top-2 -> softmax weights) computed on host with the
    exact same jnp ops as the reference, so top-k decisions match bit-for-bit.
  - Tokens are gathered per expert on host into one flat dispatch list
    shared by all cores.
  - Every core processes ALL experts' token lists, but only a 1/8 slice of
    each expert's hidden units (h1 cols [256c:256c+256) paired with the
    matching h2 cols, and the matching W2 rows). The hidden split commutes
    with swiglu, so partial outputs sum exactly. Per-core work is exactly
    sum(n_e)/8 token-equivalents — perfectly balanced, no padding (the
    matmul moving dim takes arbitrary tile sizes).
  - On-device dataflow: features on partitions / tokens on the moving free
    dim; H^T = W1ᵀ·X^T, swiglu, Y^T = W2ᵀ·S^T — no on-chip transposes.
    bf16 matmuls, fp32 accumulate, fp16 partial-y output.
  - fc2 is software-pipelined one tile behind fc1 (PE order fc1_i, fc2_{i-1})
    so the epilogue engines and late-arriving fc2 weights never stall the PE.
  - DMA queues: sync (SP HWDGE) carries b1 + the x token stream (k-granular
    for the first two tiles so the cold-start matmuls stream at DMA pace);
    scalar (ACT HWDGE) carries all weights (first expert k-granular, the
    rest trickled one issue per tile iteration so the scalar instruction
    stream is never blocked ahead of its activations); gpsimd (SWDGE)
    carries the steady-state y stores. The last two tiles store m2-granular
    on sync/scalar to cut the end-of-kernel drain.
  - Host combines: out[token] += w_k * (sum_c y_c + b2[e]).
"""

import numpy as np
import ml_dtypes

E = 8
K = 2
DIM = 1024
HID = 2048
H2 = 2 * HID  # fc1 output width (4096)
P = 128
KO1 = DIM // P  # 8 k-tiles for fc1
SH = HID // 8  # 256 hidden units per core-shard
SW = 2 * SH  # 512 fc1 output cols per shard (h1 half + h2 half)
MO1 = SW // P  # 4 m-tiles for fc1 shard output (0,1 = h1; 2,3 = h2)
KO2 = SH // P  # 2 k-tiles for fc2 shard
MO2 = DIM // P  # 8 m-tiles for fc2 output
TN = 512  # max token tile (matmul moving free dim)

_cache: dict = {}

# Extra kwargs splatted into run_bass_kernel_spmd (test harness sets this to
# enable NTFF tracing; empty by default so grading runs are unaffected).
TRACE_OPTS: dict = {}
LAST_RESULTS = None


def _tile_list(counts):
    """[(expert, tn), ...] covering each expert's token list.

    Each expert's tokens are split into ceil(ne/TN) near-equal chunks (not
    TN,TN,...,remainder) so no tile has a tiny moving dim — every matmul
    stays long enough (>=~233 rows) to hide its LDWEIGHTS behind streaming.
    Experts are ordered largest-chunk-first so the final tile (whose store
    drain is exposed) is the globally smallest.
    """
    sizes = {}
    for e, ne in enumerate(counts):
        if ne == 0:
            continue
        nchunk = -(-ne // TN)
        base, rem = divmod(ne, nchunk)
        sizes[e] = [base + (1 if i < rem else 0) for i in range(nchunk)]
    order = sorted(sizes, key=lambda e: -sizes[e][0])
    tiles = []
    for e in order:
        for tn in sizes[e]:
            tiles.append((e, tn))
    return tiles


def _build(counts):
    """Build + compile the SPMD Bass program for per-expert token counts."""
    import concourse.mybir as mybir
    import concourse.tile as tile
    from concourse import bacc
    from contextlib import ExitStack

    dt = mybir.dt
    AF = mybir.ActivationFunctionType
    ALU = mybir.AluOpType

    tiles = _tile_list(counts)
    ntiles = len(tiles)

    nc = bacc.Bacc("TRN2", target_bir_lowering=False, debug=False, num_devices=8)

    xt = nc.dram_tensor(
        "xt", [ntiles, P, KO1 * TN], dt.bfloat16, kind="ExternalInput"
    ).ap()
    w1 = nc.dram_tensor("w1", [E, P, KO1 * SW], dt.bfloat16, kind="ExternalInput").ap()
    b1 = nc.dram_tensor("b1", [P, E * MO1], dt.float32, kind="ExternalInput").ap()
    w2 = nc.dram_tensor("w2", [E, P, KO2 * DIM], dt.bfloat16, kind="ExternalInput").ap()
    yt = nc.dram_tensor(
        "yt", [ntiles, P, MO2 * TN], dt.float16, kind="ExternalOutput"
    ).ap()

    with tile.TileContext(nc) as tc, ExitStack() as ctx:
        wpool = ctx.enter_context(tc.tile_pool(name="weights", bufs=1))
        xpool = ctx.enter_context(tc.tile_pool(name="xp", bufs=6))
        spool = ctx.enter_context(tc.tile_pool(name="sp", bufs=3))
        opool = ctx.enter_context(tc.tile_pool(name="op", bufs=3))
        tpool = ctx.enter_context(tc.tile_pool(name="tp", bufs=4))
        pspool = ctx.enter_context(tc.tile_pool(name="ps", bufs=4, space="PSUM"))
        pypool = ctx.enter_context(tc.tile_pool(name="py", bufs=4, space="PSUM"))

        w1_sb = wpool.tile([P, E, KO1, SW], dt.bfloat16)
        w2_sb = wpool.tile([P, E, KO2, DIM], dt.bfloat16)
        b1_sb = wpool.tile([P, E * MO1], dt.float32)

        # PE warmup: short junk matmuls bridge the Tile preamble / first-DMA
        # window so the HAM clock gate opens before real work. The memset
        # runs on gpsimd, whose preamble slot frees earliest.
        warm_sb = wpool.tile([P, 256], dt.bfloat16)
        nc.gpsimd.memset(warm_sb[:], 0.0)
        warm_ps = pypool.tile([P, TN], dt.float32, tag="py")
        for _ in range(7):
            nc.tensor.matmul(
                warm_ps[:, :256],
                lhsT=warm_sb[:, :P],
                rhs=warm_sb[:],
                start=True,
                stop=True,
            )

        exp_order = list(dict.fromkeys(e for e, _ in tiles))
        e0 = exp_order[0]

        # x stream. DMA chunks must keep per-partition lines >= 2KB — finer
        # slices (1KB lines) measured ~2.5x slower per queue. Tile 0 goes in
        # quarters (2KB lines) on sync for the earliest possible first
        # matmul; tiles 1/3 ride the otherwise-idle gpsimd (SWDGE) queue so
        # the sync queue's serial backlog never starves the cold start.
        nc.sync.dma_start(b1_sb[:], b1[:])
        x_tiles = {}

        def emit_x(ti, nchunks, eng):
            xx = xpool.tile([P, KO1, TN], dt.bfloat16, tag="x", name=f"x_{ti}")
            flat = xx[:].rearrange("p k n -> p (k n)")
            step = KO1 * TN // nchunks
            for c in range(nchunks):
                eng.dma_start(
                    flat[:, c * step : (c + 1) * step],
                    xt[ti, :, c * step : (c + 1) * step],
                )
            x_tiles[ti] = xx

        emit_x(0, 4, nc.sync)
        emit_x(1, 2, nc.gpsimd)
        emit_x(2, 2, nc.sync)
        emit_x(3, 2, nc.gpsimd)
        for ti in range(4, ntiles):
            emit_x(ti, 1, nc.sync)

        # Scalar (ACT) queue: first expert's weights in quarters/halves so
        # tile 0's k-loop streams off arriving chunks; remaining experts'
        # weights are trickled one dma_start per tile iteration below (the
        # scalar instruction stream stays clear of its activation work).
        w1q = KO1 * SW // 4
        for q in range(4):
            nc.scalar.dma_start(
                w1_sb[:, e0].rearrange("p k n -> p (k n)")[:, q * w1q : (q + 1) * w1q],
                w1[e0, :, q * w1q : (q + 1) * w1q],
            )
        wh = KO2 * DIM // 2
        for h in range(2):
            nc.scalar.dma_start(
                w2_sb[:, e0].rearrange("p k n -> p (k n)")[:, h * wh : (h + 1) * wh],
                w2[e0, :, h * wh : (h + 1) * wh],
            )
        pending_w = []
        for e in exp_order[1:]:
            pending_w.append((w1_sb[:, e].rearrange("p k n -> p (k n)"), w1[e]))
            pending_w.append((w2_sb[:, e].rearrange("p k n -> p (k n)"), w2[e]))

        fast = set(range(max(0, ntiles - 2), ntiles))
        s_state = {}

        def fc1(ti):
            e, tn = tiles[ti]
            x_sb = x_tiles[ti]
            pss = [
                pspool.tile([P, TN], dt.float32, tag="ps", name=f"ps_{ti}_{mi}")
                for mi in range(4)
            ]
            for k in range(KO1):
                for mi in range(4):
                    nc.tensor.matmul(
                        pss[mi][:, :tn],
                        lhsT=w1_sb[:, e, k, mi * P : (mi + 1) * P],
                        rhs=x_sb[:, k, :tn],
                        start=(k == 0),
                        stop=(k == KO1 - 1),
                    )
            s_sb = spool.tile([P, KO2, TN], dt.bfloat16, tag="s", name=f"s_{ti}")
            for mi in range(2):
                t1 = tpool.tile([P, TN], dt.float32, tag="t1")
                # t1 = silu(h1 + b1a)
                nc.scalar.activation(
                    t1[:, :tn],
                    pss[mi][:, :tn],
                    AF.Silu,
                    bias=b1_sb[:, e * MO1 + mi : e * MO1 + mi + 1],
                )
                # s = (h2 + b1b) * t1   (cast to bf16 on write)
                nc.vector.scalar_tensor_tensor(
                    s_sb[:, mi, :tn],
                    pss[2 + mi][:, :tn],
                    b1_sb[:, e * MO1 + 2 + mi : e * MO1 + 3 + mi],
                    t1[:, :tn],
                    op0=ALU.add,
                    op1=ALU.mult,
                )
            s_state[ti] = s_sb

        def fc2(ti):
            e, tn = tiles[ti]
            s_sb = s_state.pop(ti)
            o_sb = opool.tile([P, MO2, TN], dt.float16, tag="o", name=f"o_{ti}")
            yt_t = yt[ti].rearrange("p (m n) -> p m n", n=TN)
            for m2 in range(MO2):
                psy = pypool.tile([P, TN], dt.float32, tag="py", name=f"psy_{ti}_{m2}")
                for k2 in range(KO2):
                    nc.tensor.matmul(
                        psy[:, :tn],
                        lhsT=w2_sb[:, e, k2, m2 * P : (m2 + 1) * P],
                        rhs=s_sb[:, k2, :tn],
                        start=(k2 == 0),
                        stop=(k2 == KO2 - 1),
                    )
                # Alternate the psum->SBUF copies between ScalarE and VectorE:
                # a single engine can't keep up at this tile rate.
                if m2 % 2 == 0:
                    nc.scalar.copy(o_sb[:, m2, :tn], psy[:, :tn])
                else:
                    nc.vector.tensor_copy(o_sb[:, m2, :tn], psy[:, :tn])
                if ti in fast:
                    # Final tiles: m2-granular stores on the two HWDGE rings
                    # so the drain after the last matmul is one small chunk.
                    eng = nc.sync if m2 % 2 == 0 else nc.scalar
                    eng.dma_start(yt_t[:, m2, :tn], o_sb[:, m2, :tn])
            if ti not in fast:
                # Steady state: one store per tile on the gpsimd (SWDGE)
                # queue, keeping the HWDGE rings free for the x stream.
                nc.gpsimd.dma_start(yt_t[:, :, :tn], o_sb[:, :, :tn])

        for i in range(ntiles + 1):
            if i < ntiles:
                if pending_w and i >= 1:
                    dst, src = pending_w.pop(0)
                    nc.scalar.dma_start(dst, src)
                fc1(i)
            if i >= 1:
                fc2(i - 1)

    nc.compile()
    return nc


def _get_nc(counts):
    key = tuple(counts)
    if key not in _cache:
        _cache[key] = _build(counts)
    return _cache[key]


def _route(x, router_w, router_b):
    """Replicate the reference router bit-for-bit (same jnp ops, same backend)."""
    import jax
    import jax.numpy as jnp

    logits = jnp.einsum("btd,ed->bte", x, router_w) + router_b
    topk_val, topk_idx = jax.lax.top_k(logits, K)
    weights = jax.nn.softmax(topk_val, axis=-1)
    return np.asarray(topk_idx), np.asarray(weights)


def kernel(x, router_w, router_b, W1, b1, W2, b2):
    from concourse.bass_utils import run_bass_kernel_spmd

    x = np.asarray(x, dtype=np.float32)
    router_w = np.asarray(router_w, dtype=np.float32)
    router_b = np.asarray(router_b, dtype=np.float32)
    W1 = np.asarray(W1, dtype=np.float32)
    b1 = np.asarray(b1, dtype=np.float32)
    W2 = np.asarray(W2, dtype=np.float32)
    b2 = np.asarray(b2, dtype=np.float32)

    B, T, _ = x.shape
    NTOK = B * T
    x_flat = x.reshape(NTOK, DIM)

    topk_idx, topk_w = _route(x, router_w, router_b)
    topk_idx = topk_idx.reshape(NTOK, K)
    topk_w = topk_w.reshape(NTOK, K).astype(np.float32)

    # Per-expert token lists + combine weights
    idx_list, w_list = [], []
    for e in range(E):
        rows, cols = np.nonzero(topk_idx == e)
        idx_list.append(rows.astype(np.int64))
        w_list.append(topk_w[rows, cols])
    counts = [len(i) for i in idx_list]

    nc = _get_nc(counts)
    tiles = _tile_list(counts)
    ntiles = len(tiles)

    bf16 = ml_dtypes.bfloat16

    # Shared token dispatch: one tile-major array used by every core.
    xt = np.zeros((ntiles, P, KO1 * TN), bf16)
    tpos = [0] * E
    for ti, (e, tn) in enumerate(tiles):
        rows = x_flat[idx_list[e][tpos[e] : tpos[e] + tn]]  # [tn, DIM]
        tpos[e] += tn
        # [j, ko*P+p] -> [p, ko*TN+j]
        blk = rows.T.reshape(KO1, P, tn).transpose(1, 0, 2)  # [P, KO1, tn]
        xt[ti].reshape(P, KO1, TN)[:, :, :tn] = blk.astype(bf16)

    in_maps = []
    for c in range(E):
        cols = np.r_[SH * c : SH * (c + 1), HID + SH * c : HID + SH * (c + 1)]
        w1c = np.zeros((E, P, KO1 * SW), bf16)
        w2c = np.zeros((E, P, KO2 * DIM), bf16)
        b1c = np.zeros((P, E * MO1), np.float32)
        for e in range(E):
            w1s = W1[e][:, cols]  # [DIM, SW]
            w1c[e] = (
                w1s.reshape(KO1, P, SW).transpose(1, 0, 2).reshape(P, KO1 * SW)
            ).astype(bf16)
            w2s = W2[e][SH * c : SH * (c + 1)]  # [SH, DIM]
            w2c[e] = (
                w2s.reshape(KO2, P, DIM).transpose(1, 0, 2).reshape(P, KO2 * DIM)
            ).astype(bf16)
            b1c[:, e * MO1 : (e + 1) * MO1] = b1[e][cols].reshape(MO1, P).T
        in_maps.append({"xt": xt, "w1": w1c, "b1": b1c, "w2": w2c})

    res = run_bass_kernel_spmd(nc, in_maps, core_ids=list(range(E)), **TRACE_OPTS)
    global LAST_RESULTS
    LAST_RESULTS = res

    # Sum the 8 shard partials, then combine per expert.
    y_sum = res.results[0]["yt"].astype(np.float32)
    for c in range(1, E):
        y_sum += res.results[c]["yt"]
    # [ti, p, m2*TN+j] -> per-tile [tn, DIM]
    y_sum = y_sum.reshape(ntiles, P, MO2, TN).transpose(0, 3, 2, 1)

    out_flat = np.zeros((NTOK, DIM), np.float32)
    tpos = [0] * E
    for ti, (e, tn) in enumerate(tiles):
        idx = idx_list[e][tpos[e] : tpos[e] + tn]
        w = w_list[e][tpos[e] : tpos[e] + tn]
        tpos[e] += tn
        y = y_sum[ti, :tn].reshape(tn, DIM) + b2[e]
        out_flat[idx] += w[:, None] * y
    return out_flat.reshape(B, T, DIM)


# revision 9
# speedup vs baseline: 1.0136x; 1.0136x over previous
"""MoE layer (E=8 experts, top-2, swiglu FFN) on 8 Trainium2 NeuronCores.

Strategy: expert dispatch on host + tensor-parallel-over-hidden on device.
  - Router (logits -> top-2 -> softmax weights) computed on host with the
    exact same jnp ops as the reference, so top-k decisions match bit-for-bit.
  - Tokens are gathered per expert on host into one flat dispatch list
    shared by all cores.
  - Every core processes ALL experts' token lists, but only a 1/8 slice of
    each expert's hidden units (h1 cols [256c:256c+256) paired with the
    matching h2 cols, and the matching W2 rows). The hidden split commutes
    with swiglu, so partial outputs sum exactly. Per-core work is exactly
    sum(n_e)/8 token-equivalents — perfectly balanced, no padding (the
    matmul moving dim takes arbitrary tile sizes).
  - On-device dataflow: features on partitions / tokens on the moving free
    dim; H^T = W1ᵀ·X^T, swiglu, Y^T = W2ᵀ·S^T — no on-chip transposes.
    bf16 matmuls, fp32 accumulate, fp16 partial-y output.
  - fc2 is software-pipelined one tile behind fc1 (PE order fc1_i, fc2_{i-1})
    so the epilogue engines and late-arriving fc2 weights never stall the PE.
  - DMA queues: sync (SP HWDGE) carries b1 + the x token stream (k-granular
    for the first two tiles so the cold-start matmuls stream at DMA pace);
    scalar (ACT HWDGE) carries all weights (first expert k-granular, the
    rest trickled one issue per tile iteration so the scalar instruction
    stream is never blocked ahead of its activations); gpsimd (SWDGE)
    carries the steady-state y stores. The last two tiles store m2-granular
    on sync/scalar to cut the end-of-kernel drain.
  - Host combines: out[token] += w_k * (sum_c y_c + b2[e]).
"""

import numpy as np
import ml_dtypes

E = 8
K = 2
DIM = 1024
HID = 2048
H2 = 2 * HID  # fc1 output width (4096)
P = 128
KO1 = DIM // P  # 8 k-tiles for fc1
SH = HID // 8  # 256 hidden units per core-shard
SW = 2 * SH  # 512 fc1 output cols per shard (h1 half + h2 half)
MO1 = SW // P  # 4 m-tiles for fc1 shard output (0,1 = h1; 2,3 = h2)
KO2 = SH // P  # 2 k-tiles for fc2 shard
MO2 = DIM // P  # 8 m-tiles for fc2 output
TN = 512  # max token tile (matmul moving free dim)

_cache: dict = {}

# Extra kwargs splatted into run_bass_kernel_spmd (test harness sets this to
# enable NTFF tracing; empty by default so grading runs are unaffected).
TRACE_OPTS: dict = {}
LAST_RESULTS = None


def _tile_list(counts):
    """[(expert, tn), ...] covering each expert's token list.

    Each expert's tokens are split into ceil(ne/TN) near-equal chunks (not
    TN,TN,...,remainder) so no tile has a tiny moving dim — every matmul
    stays long enough (>=~233 rows) to hide its LDWEIGHTS behind streaming.
    Experts are ordered largest-chunk-first so the final tile (whose store
    drain is exposed) is the globally smallest. The first two tiles are
    halved (~256 tokens) to shrink the critical cold-start DMA bytes while
    all 8 cores contend for HBM.
    """
    sizes = {}
    for e, ne in enumerate(counts):
        if ne == 0:
            continue
        nchunk = -(-ne // TN)
        base, rem = divmod(ne, nchunk)
        sizes[e] = [base + (1 if i < rem else 0) for i in range(nchunk)]
    order = sorted(sizes, key=lambda e: -sizes[e][0])
    tiles = [(e, tn) for e in order for tn in sizes[e]]
    head, nsplit = [], min(2, max(0, len(tiles) - 2))
    for e, tn in tiles[:nsplit]:
        h = tn // 2
        head += [(e, tn - h), (e, h)]
    return head + tiles[nsplit:]


def _build(counts):
    """Build + compile the SPMD Bass program for per-expert token counts."""
    import concourse.mybir as mybir
    import concourse.tile as tile
    from concourse import bacc
    from contextlib import ExitStack

    dt = mybir.dt
    AF = mybir.ActivationFunctionType
    ALU = mybir.AluOpType

    tiles = _tile_list(counts)
    ntiles = len(tiles)

    nc = bacc.Bacc("TRN2", target_bir_lowering=False, debug=False, num_devices=8)

    xt = nc.dram_tensor(
        "xt", [ntiles, P, KO1 * TN], dt.bfloat16, kind="ExternalInput"
    ).ap()
    w1 = nc.dram_tensor("w1", [E, P, KO1 * SW], dt.bfloat16, kind="ExternalInput").ap()
    b1 = nc.dram_tensor("b1", [P, E * MO1], dt.float32, kind="ExternalInput").ap()
    w2 = nc.dram_tensor("w2", [E, P, KO2 * DIM], dt.bfloat16, kind="ExternalInput").ap()
    yt = nc.dram_tensor(
        "yt", [ntiles, P, MO2 * TN], dt.float16, kind="ExternalOutput"
    ).ap()

    with tile.TileContext(nc) as tc, ExitStack() as ctx:
        wpool = ctx.enter_context(tc.tile_pool(name="weights", bufs=1))
        xpool = ctx.enter_context(tc.tile_pool(name="xp", bufs=6))
        spool = ctx.enter_context(tc.tile_pool(name="sp", bufs=3))
        opool = ctx.enter_context(tc.tile_pool(name="op", bufs=3))
        tpool = ctx.enter_context(tc.tile_pool(name="tp", bufs=4))
        pspool = ctx.enter_context(tc.tile_pool(name="ps", bufs=4, space="PSUM"))
        pypool = ctx.enter_context(tc.tile_pool(name="py", bufs=4, space="PSUM"))

        w1_sb = wpool.tile([P, E, KO1, SW], dt.bfloat16)
        w2_sb = wpool.tile([P, E, KO2, DIM], dt.bfloat16)
        b1_sb = wpool.tile([P, E * MO1], dt.float32)

        # PE warmup: short junk matmuls bridge the Tile preamble / first-DMA
        # window so the HAM clock gate opens before real work. The memset
        # runs on gpsimd, whose preamble slot frees earliest.
        warm_sb = wpool.tile([P, 256], dt.bfloat16)
        nc.gpsimd.memset(warm_sb[:], 0.0)
        warm_ps = pypool.tile([P, TN], dt.float32, tag="py")
        for _ in range(16):
            nc.tensor.matmul(
                warm_ps[:, :256],
                lhsT=warm_sb[:, :P],
                rhs=warm_sb[:],
                start=True,
                stop=True,
            )

        exp_order = list(dict.fromkeys(e for e, _ in tiles))
        e0 = exp_order[0]

        # x stream. Tiles are stored COMPACT in DRAM (first KO1*tn columns,
        # k-major) so a half-tile transfer keeps per-partition lines >= 4KB
        # — finer slices (1-2KB lines) measured 2.5-4x slower per queue.
        # Early odd tiles ride the otherwise-idle gpsimd (SWDGE) queue so
        # the sync queue's serial backlog never starves the cold start.
        nc.gpsimd.dma_start(b1_sb[:], b1[:])
        x_tiles = {}

        def emit_x(ti, nchunks, eng):
            tn = tiles[ti][1]
            xx = xpool.tile([P, KO1, TN], dt.bfloat16, tag="x", name=f"x_{ti}")
            flat = xx[:].rearrange("p k n -> p (k n)")
            step = KO1 * tn // nchunks
            for c in range(nchunks):
                eng.dma_start(
                    flat[:, c * step : (c + 1) * step],
                    xt[ti, :, c * step : (c + 1) * step],
                )
            x_tiles[ti] = xx

        for ti in range(ntiles):
            if ti in (1, 3, 5):
                emit_x(ti, 1, nc.gpsimd)
            else:
                emit_x(ti, 2 if ti < 6 else 1, nc.sync)

        # Scalar (ACT) queue: first expert's weights in halves so tile 0's
        # k-loop streams off arriving chunks; remaining experts' weights are
        # trickled one dma_start per tile iteration below (the scalar
        # instruction stream stays clear of its activation work).
        w1h = KO1 * SW // 2
        for h in range(2):
            nc.scalar.dma_start(
                w1_sb[:, e0].rearrange("p k n -> p (k n)")[:, h * w1h : (h + 1) * w1h],
                w1[e0, :, h * w1h : (h + 1) * w1h],
            )
        nc.scalar.dma_start(w2_sb[:, e0].rearrange("p k n -> p (k n)"), w2[e0])
        pending_w = []
        for e in exp_order[1:]:
            pending_w.append((w1_sb[:, e].rearrange("p k n -> p (k n)"), w1[e]))
            pending_w.append((w2_sb[:, e].rearrange("p k n -> p (k n)"), w2[e]))

        fast = set(range(max(0, ntiles - 2), ntiles))
        s_state = {}

        def fc1(ti):
            e, tn = tiles[ti]
            x_flat = x_tiles[ti][:].rearrange("p k n -> p (k n)")
            pss = [
                pspool.tile([P, TN], dt.float32, tag="ps", name=f"ps_{ti}_{mi}")
                for mi in range(4)
            ]
            for k in range(KO1):
                for mi in range(4):
                    nc.tensor.matmul(
                        pss[mi][:, :tn],
                        lhsT=w1_sb[:, e, k, mi * P : (mi + 1) * P],
                        rhs=x_flat[:, k * tn : (k + 1) * tn],
                        start=(k == 0),
                        stop=(k == KO1 - 1),
                    )
            s_sb = spool.tile([P, KO2, TN], dt.bfloat16, tag="s", name=f"s_{ti}")
            for mi in range(2):
                t1 = tpool.tile([P, TN], dt.float32, tag="t1")
                # t1 = silu(h1 + b1a)
                nc.scalar.activation(
                    t1[:, :tn],
                    pss[mi][:, :tn],
                    AF.Silu,
                    bias=b1_sb[:, e * MO1 + mi : e * MO1 + mi + 1],
                )
                # s = (h2 + b1b) * t1   (cast to bf16 on write)
                nc.vector.scalar_tensor_tensor(
                    s_sb[:, mi, :tn],
                    pss[2 + mi][:, :tn],
                    b1_sb[:, e * MO1 + 2 + mi : e * MO1 + 3 + mi],
                    t1[:, :tn],
                    op0=ALU.add,
                    op1=ALU.mult,
                )
            s_state[ti] = s_sb

        def fc2(ti):
            e, tn = tiles[ti]
            s_sb = s_state.pop(ti)
            o_sb = opool.tile([P, MO2, TN], dt.float16, tag="o", name=f"o_{ti}")
            o_flat = o_sb[:].rearrange("p m n -> p (m n)")
            for m2 in range(MO2):
                psy = pypool.tile([P, TN], dt.float32, tag="py", name=f"psy_{ti}_{m2}")
                for k2 in range(KO2):
                    nc.tensor.matmul(
                        psy[:, :tn],
                        lhsT=w2_sb[:, e, k2, m2 * P : (m2 + 1) * P],
                        rhs=s_sb[:, k2, :tn],
                        start=(k2 == 0),
                        stop=(k2 == KO2 - 1),
                    )
                # Alternate the psum->SBUF copies between ScalarE and VectorE:
                # a single engine can't keep up at this tile rate.
                if m2 % 2 == 0:
                    nc.scalar.copy(o_flat[:, m2 * tn : (m2 + 1) * tn], psy[:, :tn])
                else:
                    nc.vector.tensor_copy(
                        o_flat[:, m2 * tn : (m2 + 1) * tn], psy[:, :tn]
                    )
                if ti in fast and m2 in (3, MO2 - 1):
                    # Final tiles: half-tile stores on the two HWDGE rings so
                    # the drain after the last matmul is one small chunk.
                    eng = nc.sync if m2 == 3 else nc.scalar
                    lo, hi = (0, 4 * tn) if m2 == 3 else (4 * tn, MO2 * tn)
                    eng.dma_start(yt[ti, :, lo:hi], o_flat[:, lo:hi])
            if ti not in fast:
                # Steady state: one compact store per tile on the gpsimd
                # (SWDGE) queue, keeping the HWDGE rings free for the x
                # stream. Compact layout keeps per-partition lines >= 6KB.
                nc.gpsimd.dma_start(yt[ti, :, : MO2 * tn], o_flat[:, : MO2 * tn])

        for i in range(ntiles + 1):
            if i < ntiles:
                if pending_w and i >= 1:
                    dst, src = pending_w.pop(0)
                    nc.scalar.dma_start(dst, src)
                fc1(i)
            if i >= 1:
                fc2(i - 1)

    nc.compile()
    return nc


def _get_nc(counts):
    key = tuple(counts)
    if key not in _cache:
        _cache[key] = _build(counts)
    return _cache[key]


def _route(x, router_w, router_b):
    """Replicate the reference router bit-for-bit (same jnp ops, same backend)."""
    import jax
    import jax.numpy as jnp

    logits = jnp.einsum("btd,ed->bte", x, router_w) + router_b
    topk_val, topk_idx = jax.lax.top_k(logits, K)
    weights = jax.nn.softmax(topk_val, axis=-1)
    return np.asarray(topk_idx), np.asarray(weights)


def kernel(x, router_w, router_b, W1, b1, W2, b2):
    from concourse.bass_utils import run_bass_kernel_spmd

    x = np.asarray(x, dtype=np.float32)
    router_w = np.asarray(router_w, dtype=np.float32)
    router_b = np.asarray(router_b, dtype=np.float32)
    W1 = np.asarray(W1, dtype=np.float32)
    b1 = np.asarray(b1, dtype=np.float32)
    W2 = np.asarray(W2, dtype=np.float32)
    b2 = np.asarray(b2, dtype=np.float32)

    B, T, _ = x.shape
    NTOK = B * T
    x_flat = x.reshape(NTOK, DIM)

    topk_idx, topk_w = _route(x, router_w, router_b)
    topk_idx = topk_idx.reshape(NTOK, K)
    topk_w = topk_w.reshape(NTOK, K).astype(np.float32)

    # Per-expert token lists + combine weights
    idx_list, w_list = [], []
    for e in range(E):
        rows, cols = np.nonzero(topk_idx == e)
        idx_list.append(rows.astype(np.int64))
        w_list.append(topk_w[rows, cols])
    counts = [len(i) for i in idx_list]

    nc = _get_nc(counts)
    tiles = _tile_list(counts)
    ntiles = len(tiles)

    bf16 = ml_dtypes.bfloat16

    # Shared token dispatch: one tile-major array used by every core. Each
    # tile is packed COMPACT (first KO1*tn columns, k-major) so device DMA
    # chunks keep big per-partition lines.
    xt = np.zeros((ntiles, P, KO1 * TN), bf16)
    tpos = [0] * E
    for ti, (e, tn) in enumerate(tiles):
        rows = x_flat[idx_list[e][tpos[e] : tpos[e] + tn]]  # [tn, DIM]
        tpos[e] += tn
        # [j, ko*P+p] -> [p, ko*tn+j]
        blk = rows.T.reshape(KO1, P, tn).transpose(1, 0, 2)  # [P, KO1, tn]
        xt[ti, :, : KO1 * tn] = blk.reshape(P, KO1 * tn).astype(bf16)

    in_maps = []
    for c in range(E):
        cols = np.r_[SH * c : SH * (c + 1), HID + SH * c : HID + SH * (c + 1)]
        w1c = np.zeros((E, P, KO1 * SW), bf16)
        w2c = np.zeros((E, P, KO2 * DIM), bf16)
        b1c = np.zeros((P, E * MO1), np.float32)
        for e in range(E):
            w1s = W1[e][:, cols]  # [DIM, SW]
            w1c[e] = (
                w1s.reshape(KO1, P, SW).transpose(1, 0, 2).reshape(P, KO1 * SW)
            ).astype(bf16)
            w2s = W2[e][SH * c : SH * (c + 1)]  # [SH, DIM]
            w2c[e] = (
                w2s.reshape(KO2, P, DIM).transpose(1, 0, 2).reshape(P, KO2 * DIM)
            ).astype(bf16)
            b1c[:, e * MO1 : (e + 1) * MO1] = b1[e][cols].reshape(MO1, P).T
        in_maps.append({"xt": xt, "w1": w1c, "b1": b1c, "w2": w2c})

    res = run_bass_kernel_spmd(nc, in_maps, core_ids=list(range(E)), **TRACE_OPTS)
    global LAST_RESULTS
    LAST_RESULTS = res

    # Sum the 8 shard partials, then combine per expert.
    y_sum = res.results[0]["yt"].astype(np.float32)
    for c in range(1, E):
        y_sum += res.results[c]["yt"]

    out_flat = np.zeros((NTOK, DIM), np.float32)
    tpos = [0] * E
    for ti, (e, tn) in enumerate(tiles):
        idx = idx_list[e][tpos[e] : tpos[e] + tn]
        w = w_list[e][tpos[e] : tpos[e] + tn]
        tpos[e] += tn
        # compact [p, m2*tn+j] -> [tn, DIM] with dim = m2*P + p
        y = y_sum[ti, :, : MO2 * tn].reshape(P, MO2, tn).transpose(2, 1, 0)
        y = y.reshape(tn, DIM) + b2[e]
        out_flat[idx] += w[:, None] * y
    return out_flat.reshape(B, T, DIM)
